# revision 29
# baseline (speedup 1.0000x reference)
"""Masked cross-modal attention on 8 Trainium2 NeuronCores (v3).

Reference math (per batch b):
    q,k,v = x @ W{q,k,v}.T   (head-major channels, H=8, Dh=64)
    s     = (q @ k.T) / 8, masked_fill(mask==0, 1e-9), softmax over keys
    out   = (att @ v) @ Wout.T

Masked positions contribute weight exp(1e-9)=1 and value v_j independent of
the query, so attention runs only over gathered unmasked keys (~half).  The
|M| denominator correction is folded into the indicator column of the padded
(zero) key rows: a zero key row scores 0 against every query, exp gives
exactly 1, and indicator (N-cnt)/n_pad makes the indicator matmul emit the
complete denominator.  The masked-value numerator correction is a tiny
host-side [64] vector fused into the normalize multiply.

Sharding: core c -> batch c//2, head-group c%2 (4 of 8 heads).  Each core
emits two partial [2048,512] outputs (one per head-pair through its Wout
slice); the host sums the four partials per batch.

Engine layout per core: PE does QKV projections (bf16), scores (bf16),
exp-weighted value sums with indicator column (f32r), and the output
projection (f32r), software-pipelined as scores(sc) -> attV(sc-1) so PE
always has a dependency-free matmul in flight.  ACT does only exp (f32r
out) plus the second head-pair's output-tile copies at the tail.  DVE
handles PSUM evacuation and the normalize chain.  QKV and first-pair Wout
groups are drip-fed into scheduled slots of the attention loop.
"""

import sys

for _p in ("/opt/trn_rl_repo", "/root/.axon_site/_ro/trn_rl_repo"):
    if _p not in sys.path:
        sys.path.append(_p)

import numpy as np
import ml_dtypes
import concourse.bass as bass
import concourse.mybir as mybir
import concourse.tile as tile
from concourse import bacc
from concourse.bass_utils import run_bass_kernel_spmd

F32 = mybir.dt.float32
F32R = mybir.dt.float32r
BF16 = mybir.dt.bfloat16
EXP = mybir.ActivationFunctionType.Exp
ADD = mybir.AluOpType.add
MULT = mybir.AluOpType.mult

XDT = BF16                         # x / xg / Wq / Wk / Wv / Q^T / K^T
XDT_NP = ml_dtypes.bfloat16
VDT = F32R                         # e / V / att / Wout operands
VDT_NP = np.float32
ODT = BF16                         # output partials
ODT_NP = ml_dtypes.bfloat16

B, N, DIM = 4, 2048, 512
DL = 256                          # 4 heads * 64 dims per core
SCALE = 64 ** -0.5


def _build(nc, s_pad):
    n_sc = s_pad // 128

    xt = nc.dram_tensor("XT", [DIM, N], XDT, kind="ExternalInput")
    xgt = nc.dram_tensor("XGT", [DIM, s_pad], XDT, kind="ExternalInput")
    indv = nc.dram_tensor("INDV", [s_pad, 4], VDT, kind="ExternalInput")
    wqt = nc.dram_tensor("WQT", [DIM, DL], XDT, kind="ExternalInput")
    wkt = nc.dram_tensor("WKT", [DIM, DL], XDT, kind="ExternalInput")
    wvt = nc.dram_tensor("WVT", [DIM, DL], XDT, kind="ExternalInput")
    wot = nc.dram_tensor("WOT", [DL, DIM], VDT, kind="ExternalInput")
    corr = nc.dram_tensor("CORR", [64, 4], F32, kind="ExternalInput")
    outs = [nc.dram_tensor(f"OUT{hp}", [N, DIM], ODT, kind="ExternalOutput")
            for hp in range(2)]

    s_tiles = [(i * 512, min(512, s_pad - i * 512)) for i in range((s_pad + 511) // 512)]

    with tile.TileContext(nc) as tc:
        with (
            tc.tile_pool(name="persist", bufs=1) as pp,
            tc.tile_pool(name="psaux", bufs=2, space="PSUM") as psaux,
            tc.tile_pool(name="psreg", bufs=2, space="PSUM") as psreg,
            tc.tile_pool(name="pspn", bufs=1, space="PSUM") as pspn,
            tc.tile_pool(name="epool", bufs=3) as ep,
            tc.tile_pool(name="npool", bufs=2) as np_pool,
            tc.tile_pool(name="dpool", bufs=2) as dpool,
            tc.tile_pool(name="drampool", bufs=2, space="DRAM") as drp,
            tc.tile_pool(name="opool", bufs=3) as op,
        ):
            wq_sb = pp.tile([128, 4 * DL], XDT)
            wk_sb = pp.tile([128, 4 * DL], XDT)
            wv_sb = pp.tile([128, 4 * DL], XDT)
            wo_sb = pp.tile([128, 2 * DIM], VDT)
            corr_sb = pp.tile([64, 4], F32)
            qt_sb = pp.tile([128, 2 * N], XDT)           # [hp][t]
            kt_sb = pp.tile([128, 2 * s_pad], XDT)       # [hp][s]
            v_sb = pp.tile([128, n_sc * 4 * 65], VDT)    # [sc][h][65]
            att_pair = [pp.tile([128, N], VDT, name=f"attp{i}") for i in range(2)]

            ones_f32 = pp.tile([1, 64], F32)
            nc.gpsimd.memset(ones_f32[:], 1.0)
            ones_sb = pp.tile([1, 64], VDT)
            nc.vector.tensor_copy(ones_sb[:], ones_f32[:])

            # --- input DMAs: batched 3D APs, split across the two hwdge
            # queues (SP: K/V path that gates the first scores; ACT: Q path)
            xg_all = pp.tile([128, 4 * s_pad], XDT, name="xg_all")
            xg_tiles = [xg_all[:, k * s_pad:(k + 1) * s_pad] for k in range(4)]
            xt_all = pp.tile([128, 4 * N], XDT, name="xt_all")
            xt_tiles = [xt_all[:, k * N:(k + 1) * N] for k in range(4)]
            v_view = v_sb[:].rearrange("p (s h x) -> p s h x", s=n_sc, h=4)
            xt_src = xt.ap().rearrange("(k p) d -> p k d", p=128, k=4)

            def ld3(eng, dst_pkd, src, k):
                eng.dma_start(dst_pkd, src.ap().rearrange("(k p) d -> p k d", p=128, k=k))

            ld3(nc.sync, wk_sb[:].rearrange("p (k d) -> p k d", k=4), wkt, 4)
            ld3(nc.sync, xg_all[:].rearrange("p (k d) -> p k d", k=4), xgt, 4)
            ld3(nc.sync, wv_sb[:].rearrange("p (k d) -> p k d", k=4), wvt, 4)
            ind_sb = pp.tile([128, n_sc * 4], VDT, name="ind_sb")
            nc.sync.dma_start(ind_sb[:].rearrange("p (s h) -> p s h", s=n_sc),
                              indv.ap().rearrange("(s p) h -> p s h", p=128, s=n_sc))
            nc.gpsimd.tensor_copy(v_view[:, :, :, 64],
                                  ind_sb[:].rearrange("p (s h) -> p s h", s=n_sc))
            nc.sync.dma_start(corr_sb[:], corr.ap())
            ld3(nc.sync, wo_sb[:].rearrange("p (k d) -> p k d", k=2), wot, 2)
            ld3(nc.scalar, wq_sb[:].rearrange("p (k d) -> p k d", k=4), wqt, 4)
            nc.scalar.dma_start(
                xt_all[:].rearrange("p (k d) -> p k d", k=4)[:, :, 0:1024],
                xt_src[:, :, 0:1024])
            nc.scalar.dma_start(
                xt_all[:].rearrange("p (k d) -> p k d", k=4)[:, :, 1024:2048],
                xt_src[:, :, 1024:2048])

            def emit_kt(hp, si):
                s0, sw = s_tiles[si]
                pk = psaux.tile([128, 512], F32, tag="psaux", name="pk")
                for k in range(4):
                    nc.tensor.matmul(
                        pk[:, :sw],
                        wk_sb[:, k * DL + hp * 128: k * DL + (hp + 1) * 128],
                        xg_tiles[k][:, s0:s0 + sw],
                        start=(k == 0), stop=(k == 3),
                    )
                nc.vector.tensor_copy(kt_sb[:, hp * s_pad + s0: hp * s_pad + s0 + sw], pk[:, :sw])

            def emit_qt(hp, t):
                pq = psaux.tile([128, 512], F32, tag="psaux", name="pq")
                for k in range(4):
                    nc.tensor.matmul(
                        pq[:],
                        wq_sb[:, k * DL + hp * 128: k * DL + (hp + 1) * 128],
                        xt_tiles[k][:, t * 512:(t + 1) * 512],
                        start=(k == 0), stop=(k == 3),
                    )
                nc.vector.tensor_copy(qt_sb[:, hp * N + t * 512: hp * N + (t + 1) * 512], pq[:])

            def emit_v(sc):
                pv = psaux.tile([128, 256], F32, tag="psaux", name="pv")
                for k in range(4):
                    nc.tensor.matmul(
                        pv[:],
                        xg_tiles[k][:, sc * 128:(sc + 1) * 128],
                        wv_sb[:, k * DL:(k + 1) * DL],
                        start=(k == 0), stop=(k == 3),
                    )
                nc.vector.tensor_copy(
                    v_view[:, sc, :, 0:64],
                    pv[:].rearrange("p (h x) -> p h x", h=4),
                )

            deferred_out_dmas = []

            def emit_wout(hp, tcn, defer_dma=False):
                po = psaux.tile([128, 512], F32, tag="psaux", name="po")
                nc.tensor.matmul(
                    po[:],
                    att_pair[hp][:, tcn * 128:(tcn + 1) * 128],
                    wo_sb[:, hp * DIM:(hp + 1) * DIM],
                    start=True, stop=True,
                )
                o_sb = op.tile([128, 512], ODT, tag="o")
                if hp == 0 or tcn < 11:
                    nc.vector.tensor_copy(o_sb[:], po[:])
                else:
                    nc.scalar.copy(o_sb[:], po[:])
                dma = lambda: nc.sync.dma_start(
                    outs[hp].ap()[tcn * 128:(tcn + 1) * 128, :], o_sb[:])
                if defer_dma:
                    deferred_out_dmas.append(dma)
                else:
                    dma()

            def emit_normalize(hp, hl, half, numer, via_pe=False):
                gh = 2 * hp + hl
                par = hl * 64
                if via_pe:
                    # broadcast 1/den across partitions with a ones-matmul
                    # (PE + PSUM are free at the tail; skips the DMA queue)
                    rbf = dpool.tile([1, 1024], F32, tag="rb1")
                    nc.vector.reciprocal_approx_fast(out=rbf[:], in_=numer[64:65, :])
                    rbt = dpool.tile([1, 1024], VDT, tag="rb1r")
                    nc.vector.tensor_copy(rbt[:], rbf[:])
                    rbr = rbt[:]
                    rps = psreg.tile([64, 1024], F32, tag="reg", name="rps")
                    for j in range(2):
                        nc.tensor.matmul(rps[:, j * 512:(j + 1) * 512], ones_sb[:],
                                         rbr[:, j * 512:(j + 1) * 512], start=True, stop=True)
                    rbc = rps
                else:
                    scratch = drp.tile([1024], F32, tag="scr")
                    nc.sync.dma_start(scratch[:].unsqueeze(0), numer[64:65, :])
                    bden = dpool.tile([64, 1024], F32, tag="bden")
                    nc.sync.dma_start(bden[:], scratch[:].unsqueeze(0).broadcast_to([64, 1024]))
                    rbc = dpool.tile([64, 1024], F32, tag="rbc")
                    nc.vector.reciprocal_approx_fast(out=rbc[:], in_=bden[:])
                nc.vector.scalar_tensor_tensor(
                    out=att_pair[hp][par:par + 64, half * 1024:(half + 1) * 1024],
                    in0=numer[0:64, :],
                    scalar=corr_sb[:, gh:gh + 1],
                    in1=rbc[:],
                    op0=ADD, op1=MULT,
                )

            # fillers[unit][slot] -> list of closures to emit in that sc slot;
            # unit = (hp, hl, half) flattened 0..7, slot = sc index 1..n_sc-1
            fillers = {}

            def add_fill(unit, slot, fn):
                fillers.setdefault((unit, slot), []).append(fn)

            # unit 0 (h0-half0): rest of kt hp0, v4.., qt02/03 before half1
            add_fill(0, 1, lambda: emit_kt(0, 1))
            if len(s_tiles) > 2:
                add_fill(0, 2, lambda: emit_kt(0, 2))
            for sc in range(4, n_sc):
                add_fill(0, min(sc - 1, n_sc - 2), lambda sc=sc: emit_v(sc))
            add_fill(0, n_sc - 2, lambda: emit_qt(0, 2))
            add_fill(0, n_sc - 1, lambda: emit_qt(0, 3))
            # unit 1 (h0-half1): kt/qt of hp1
            for i, _ in enumerate(s_tiles):
                add_fill(1, 1 + i, lambda i=i: emit_kt(1, i))
            for t in range(4):
                add_fill(1, 4 + t, lambda t=t: emit_qt(1, t))
            # units 4,5 (h2): Wout hp0 chunks
            for i in range(16):
                add_fill(4 + i // 8, 1 + (i % 8), lambda i=i: emit_wout(0, i))
            # unit 7 (h3-half1): first Wout hp1 chunks once h3-half0's
            # normalize chain (~5us latency) has surely landed
            for i in range(6):
                add_fill(7, 6 + i // 2, lambda i=i: emit_wout(1, i, defer_dma=True))

            # head start: kt first (its DMAs land first), q, then v
            emit_kt(0, 0)
            emit_qt(0, 0)
            emit_qt(0, 1)
            for sc in range(4):
                emit_v(sc)

            for hp in range(2):
                for hl in range(2):
                    gh = 2 * hp + hl
                    par = hl * 64
                    for half in range(2):
                        unit = ((hp * 2) + hl) * 2 + half
                        pn = pspn.tile([65, 1024], F32, tag="pn")
                        e_tiles = {}
                        for sc in range(n_sc):
                            reg = psreg.tile([128, 1024], F32, tag="reg")
                            for j in range(2):
                                nc.tensor.matmul(
                                    reg[:, j * 512:(j + 1) * 512],
                                    kt_sb[par:par + 64, hp * s_pad + sc * 128: hp * s_pad + (sc + 1) * 128],
                                    qt_sb[par:par + 64,
                                          hp * N + half * 1024 + j * 512: hp * N + half * 1024 + (j + 1) * 512],
                                    start=True, stop=True,
                                )
                            e_sb = ep.tile([128, 1024], VDT, tag="e")
                            nc.scalar.activation(e_sb[:], reg[:], EXP, scale=SCALE)
                            e_tiles[sc] = e_sb
                            for fn in fillers.pop((unit, sc), ()):
                                fn()
                            if sc > 0:
                                prev = e_tiles.pop(sc - 1)
                                for j in range(2):
                                    nc.tensor.matmul(
                                        pn[:, j * 512:(j + 1) * 512],
                                        v_sb[:, ((sc - 1) * 4 + gh) * 65:((sc - 1) * 4 + gh + 1) * 65],
                                        prev[:, j * 512:(j + 1) * 512],
                                        start=(sc - 1 == 0), stop=False,
                                    )
                        last = e_tiles.pop(n_sc - 1)
                        for j in range(2):
                            nc.tensor.matmul(
                                pn[:, j * 512:(j + 1) * 512],
                                v_sb[:, ((n_sc - 1) * 4 + gh) * 65:((n_sc - 1) * 4 + gh + 1) * 65],
                                last[:, j * 512:(j + 1) * 512],
                                start=False, stop=True,
                            )
                        numer = np_pool.tile([65, 1024], F32, tag="numer")
                        nc.vector.tensor_copy(numer[:], pn[:])
                        emit_normalize(hp, hl, half, numer)
                        if unit == 7:
                            for dma in deferred_out_dmas:
                                dma()
            assert not fillers, f"unconsumed fillers: {list(fillers)}"
            for tcn in range(6, 16):
                emit_wout(1, tcn)

    nc.compile()
    return nc


def _prep(input_feature, mask, Wq, Wk, Wv, Wout):
    x = np.ascontiguousarray(np.asarray(input_feature, dtype=np.float32))
    m = np.asarray(mask)
    Wq = np.asarray(Wq, dtype=np.float32)
    Wk = np.asarray(Wk, dtype=np.float32)
    Wv = np.asarray(Wv, dtype=np.float32)
    Wout = np.asarray(Wout, dtype=np.float32)

    idxs = [np.flatnonzero(m[b]) for b in range(B)]
    max_cnt = max(len(i) for i in idxs)
    s_pad = max(128, ((max_cnt + 127) // 128) * 128)
    if s_pad == max_cnt:
        s_pad += 128  # every batch needs >=1 phantom key row

    in_maps = []
    for c in range(8):
        b, g = c // 2, c % 2
        idx = idxs[b]
        cnt = len(idx)
        xg = np.zeros((s_pad, DIM), np.float32)
        xg[:cnt] = x[b][idx]
        n_pad = s_pad - cnt
        iv = np.zeros((s_pad, 4), np.float32)
        iv[:cnt] = 1.0
        iv[cnt:] = np.float32(N - cnt) / np.float32(n_pad)
        xm = x[b][m[b] == 0].sum(axis=0, dtype=np.float32)
        corr = np.zeros((64, 4), np.float32)
        for h in range(4):
            hg = g * 4 + h
            corr[:, h] = Wv[hg * 64:(hg + 1) * 64, :] @ xm
        in_maps.append({
            "XT": np.ascontiguousarray(x[b].T.astype(XDT_NP)),
            "XGT": np.ascontiguousarray(xg.T.astype(XDT_NP)),
            "INDV": np.ascontiguousarray(iv.astype(VDT_NP)),
            "WQT": np.ascontiguousarray(Wq[g * DL:(g + 1) * DL, :].T.astype(XDT_NP)),
            "WKT": np.ascontiguousarray(Wk[g * DL:(g + 1) * DL, :].T.astype(XDT_NP)),
            "WVT": np.ascontiguousarray(Wv[g * DL:(g + 1) * DL, :].T.astype(XDT_NP)),
            "WOT": np.ascontiguousarray(Wout[:, g * DL:(g + 1) * DL].T.astype(VDT_NP)),
            "CORR": corr,
        })
    return in_maps, s_pad


def _run(in_maps, s_pad, trace=False):
    nc = bacc.Bacc("TRN2", target_bir_lowering=False, debug=False, num_devices=8)
    _build(nc, s_pad)
    res = run_bass_kernel_spmd(nc, in_maps, core_ids=list(range(8)), trace=trace)
    out = np.empty((B, N, DIM), np.float32)
    for b in range(B):
        out[b] = (res.results[2 * b]["OUT0"].astype(np.float32)
                  + res.results[2 * b]["OUT1"].astype(np.float32)
                  + res.results[2 * b + 1]["OUT0"].astype(np.float32)
                  + res.results[2 * b + 1]["OUT1"].astype(np.float32))
    return out, res


def kernel(input_feature, mask, Wq, Wk, Wv, Wout):
    in_maps, s_pad = _prep(input_feature, mask, Wq, Wk, Wv, Wout)
    out, _ = _run(in_maps, s_pad)
    return out
